# revision 22
# baseline (speedup 1.0000x reference)
# Bass/Trainium2 SPMD kernel for nn_AdaptiveIncrementalAttention.
#
# Math (per batch b):
#   q = W_q @ e_i[b]                                     [D]   (host-staged)
#   k[n, e] = sum_d e_j[b, n, d] * W_k[e, d]             [N, D]
#   sigma[n] = sum_e omega[e] * tanh(q[e] + k[n, e])     [N]
#   a[n] = imp[n] * exp(sigma[n]) / (sum_n exp(sigma) + 1e-9)
#   u[d] = sum_n a[n] * e_j[b, n, d]                     [D]   (weighted sum
#   A[e] = sum_d u[d] * W_v[e, d]                        [D]    commutes with W_v)
#   A_lk[l, d] = A[d] * R_lk[l, d]                       [L, D] (outer broadcast)
#   returns (A, A_lk, A)  -- A_l is identical to A in the source module.
#
# Sharding: data-parallel over batch B=8 across the 8 NeuronCores; weights
# replicated, no collectives. Host pre-transposes W_k/W_v and the e_j shards
# into the d-major layouts the PE array contracts over (layout staging only).
# The N x D projection runs as float32r (full-rate fp32 PE mode); softmax /
# weighted-sum pipeline overlaps it tile by tile.
import sys
import os

sys.path.insert(0, "/opt/trn_rl_repo")
import numpy as np

B, N, D, L = 8, 2048, 1024, 128
P = 128
NT = N // P  # 16 n-tiles
DC = D // P  # 8 d-chunks
H = D // 512  # 2 psum halves per row block
EG = 4  # eT tiles per DMA group
NG = NT // EG  # 4 groups
ROWS = D + N + P  # packed row-vector input: qrow | imp | ones

_CACHE = {}


def _build_nc():
    import concourse.mybir as mybir
    import concourse.tile as tile
    from concourse import bacc

    F32 = mybir.dt.float32
    F32R = mybir.dt.float32r
    BF16 = mybir.dt.bfloat16
    AF = mybir.ActivationFunctionType
    OP = mybir.AluOpType
    AX = mybir.AxisListType

    nc = bacc.Bacc(None, target_bir_lowering=False, debug=False)

    # Per-core inputs (host-staged layouts; see _stage_inputs below).
    eTs = nc.declare_dram_parameter("eTs", [NG, P, EG, DC, P], BF16, isOutput=False)
    wkT = nc.declare_dram_parameter("wkT", [P, DC, D], BF16, isOutput=False)
    wvT = nc.declare_dram_parameter("wvT", [P, DC, D], F32R, isOutput=False)
    rowsp = nc.declare_dram_parameter("rowsp", [1, ROWS], F32R, isOutput=False)
    planes = nc.declare_dram_parameter("planes", [P, P + D], F32, isOutput=False)
    rlk = nc.declare_dram_parameter("rlk", [L, D], F32, isOutput=False)
    A_out = nc.declare_dram_parameter("A_out", [1, D], F32, isOutput=True)
    Alk_out = nc.declare_dram_parameter("Alk_out", [L, D], F32, isOutput=True)

    with tile.TileContext(nc) as tc:
        with (
            tc.tile_pool(name="per", bufs=1) as per,
            tc.tile_pool(name="et", bufs=3) as etp,
            tc.tile_pool(name="tt", bufs=2) as ttp,
            tc.tile_pool(name="scr", bufs=2) as scr,
            tc.tile_pool(name="kps", bufs=6, space="PSUM") as kps,
            tc.tile_pool(name="wps", bufs=2, space="PSUM") as wps,
        ):
            PE, ACT, DVE = nc.tensor, nc.scalar, nc.vector

            # Persistent SBUF residents.
            wk_sb = per.tile([P, DC, D], BF16, tag="wk")
            wv_sb = per.tile([P, DC, D], F32R, tag="wv")
            rows_sb = per.tile([1, ROWS], F32R, tag="rows")
            planes_sb = per.tile([P, P + D], F32, tag="planes")
            rlk_sb = per.tile([L, D], F32, tag="rlk")
            sig_sb = per.tile([P, NT], F32, tag="sig")
            exr_sb = per.tile([1, N], F32, tag="exr")
            wrf_sb = per.tile([1, N], BF16, tag="wrf")
            sxp_sb = per.tile([1, NT], F32, tag="sxp")
            utp_sb = per.tile([P, DC * NT], F32, tag="utp")
            ut_sb = per.tile([P, DC], F32, tag="ut")
            uta_sb = per.tile([P, DC], F32, tag="uta")
            utr_sb = per.tile([P, DC], F32R, tag="utr")
            A_sb = per.tile([1, D], F32R, tag="Asb")
            Af_sb = per.tile([1, D], F32, tag="Afsb")
            Ab_sb = per.tile([P, D], F32, tag="Absb")
            alk_sb = per.tile([L, D], F32, tag="alk")
            S_sb = per.tile([1, 1], F32, tag="S")
            inv_sb = per.tile([1, 1], F32, tag="inv")
            invb_sb = per.tile([P, 1], F32, tag="invb")
            onesbf_sb = per.tile([1, P], BF16, tag="onesbf")

            qrow = rows_sb[0:1, 0:D]
            impr = rows_sb[0:1, D : D + N].bitcast(F32)
            onesr = rows_sb[0:1, D + N : D + N + P]
            idnt = planes_sb[:, 0:P]
            omgb = planes_sb[:, P : P + D]

            # Head DMAs: small packs + first eT group + W_k halves, ordered so
            # tile 0's operands land first. eT groups stream on the scalar
            # (ACT) HWDGE so descriptor generation runs parallel to sync's.
            nc.sync.dma_start(rows_sb[:], rowsp[:])
            et_s = {}
            et_g = {}
            for i in range(EG):
                et_s[i] = etp.tile([P, DC, P], BF16, tag="et1", name=f"ets{i}", bufs=4)
            nc.sync.dma_start(et_s[0][:], eTs[0][:, 0, :, :])
            nc.sync.dma_start(wk_sb[:, 0:4, :], wkT[:, 0:4, :])
            nc.sync.dma_start(wk_sb[:, 4:8, :], wkT[:, 4:8, :])
            nc.scalar.dma_start(et_s[1][:], eTs[0][:, 1, :, :])
            nc.scalar.dma_start(et_s[2][:], eTs[0][:, 2, :, :])
            nc.scalar.dma_start(et_s[3][:], eTs[0][:, 3, :, :])
            nc.sync.dma_start(planes_sb[:], planes[:])

            ACT.activation(onesbf_sb[0:1, :], onesr.bitcast(F32), AF.Copy)

            def et_tile(i):
                if i < EG:
                    return et_s[i][:, :, :]
                return et_g[i // EG][:, i % EG, :, :]

            kp_tiles = {}
            wb_tiles = {}
            pb_insts = {}

            def emit_dmms(i):
                kph = [kps.tile([P, 512], F32, tag="k", name=f"kp{i}_{h}") for h in range(H)]
                kp_tiles[i] = kph
                et = et_tile(i)
                for c in range(DC):
                    for h in range(H):
                        PE.matmul(
                            kph[h][:, :],
                            et[:, c, :],
                            wk_sb[:, c, h * 512 : (h + 1) * 512],
                            start=(c == 0),
                            stop=False,
                        )

            def emit_rank1(i):
                # Close the accumulation with k += ones ⊗ q_row (the q bias).
                for h in range(H):
                    PE.matmul(
                        kp_tiles[i][h][:, :],
                        onesr,
                        qrow[0:1, h * 512 : (h + 1) * 512],
                        start=False,
                        stop=True,
                    )

            def emit_tanh_sig(i):
                t = ttp.tile([P, D], F32, tag="t")
                for h in range(H):
                    ACT.activation(t[:, h * 512 : (h + 1) * 512], kp_tiles[i][h][:, :], AF.Tanh)
                s = scr.tile([P, D], BF16, tag="sbf")
                DVE.scalar_tensor_tensor(
                    out=s[:],
                    in0=t[:],
                    scalar=1.0,
                    in1=omgb,
                    op0=OP.mult,
                    op1=OP.mult,
                    accum_out=sig_sb[:, i : i + 1],
                )

            def emit_wpath(i):
                # sigma col -> row, exp (with per-tile sum), w = imp * exp,
                # then broadcast w across partitions for the u-accumulation.
                srp = wps.tile([1, P], F32, tag="w")
                PE.transpose(srp[:], sig_sb[:, i : i + 1], idnt)
                ACT.activation(
                    exr_sb[0:1, i * P : (i + 1) * P],
                    srp[0:1, :],
                    AF.Exp,
                    accum_out=sxp_sb[0:1, i : i + 1],
                )
                DVE.tensor_tensor(
                    out=wrf_sb[0:1, i * P : (i + 1) * P],
                    in0=exr_sb[0:1, i * P : (i + 1) * P],
                    in1=impr[0:1, i * P : (i + 1) * P],
                    op=OP.mult,
                )
                wrep = (
                    wrf_sb[0:1, i * P : (i + 1) * P]
                    .rearrange("o (x j) -> o x j", x=1)
                    .broadcast_to([1, EG, P])
                )
                wbp = [wps.tile([P, 512], F32, tag="w", name=f"wbp{i}_{h}") for h in range(H)]
                for h in range(H):
                    pb = PE.matmul(wbp[h][:, :], onesbf_sb[0:1, :], wrep, start=True, stop=True)
                wb = scr.tile([P, D], BF16, tag="wb")
                wb_tiles[i] = wb
                for h in range(H):
                    ACT.activation(
                        wb[:, h * 512 : (h + 1) * 512], wbp[h][:, :], AF.Copy
                    )
                pb_insts[i] = pb

            def emit_ut(i):
                # us = eT_tile * w8 (flat 2-D bf16 for the fast DVE mode),
                # then per-chunk row sums into utp[:, i*DC:(i+1)*DC].
                us = scr.tile([P, DC * P], BF16, tag="us")
                et = et_tile(i).rearrange("p c j -> p (c j)")
                DVE.tensor_tensor(out=us[:], in0=et, in1=wb_tiles[i][:], op=OP.mult)
                DVE.tensor_reduce(
                    utp_sb[:, i * DC : (i + 1) * DC],
                    us[:].rearrange("p (c j) -> p c j", c=DC),
                    axis=AX.X,
                    op=OP.add,
                )

            # Software-pipelined main loop: rank1/tanh lag 2, w/u lag 3 so the
            # PE stream never waits on the ACT/DVE chain of the same tile.
            for i in range(NT + 3):
                if i < NT:
                    emit_dmms(i)
                if i == 0:
                    et_g[1] = etp.tile([P, EG, DC, P], BF16, tag="et", name="etg1")
                    nc.scalar.dma_start(et_g[1][:], eTs[1])
                if i == 3:
                    et_g[2] = etp.tile([P, EG, DC, P], BF16, tag="et", name="etg2")
                    nc.scalar.dma_start(et_g[2][:], eTs[2])
                if i == 10:
                    et_g[3] = etp.tile([P, EG, DC, P], BF16, tag="et", name="etg3")
                    nc.scalar.dma_start(et_g[3][:], eTs[3])
                if i == 6:
                    from concourse.tile_rust import add_dep_helper

                    d1 = nc.gpsimd.dma_start(wv_sb[:], wvT[:])
                    d2 = nc.gpsimd.dma_start(rlk_sb[:], rlk[:])
                    anchor = pb_insts[2]
                    add_dep_helper(
                        d1.ins, anchor.ins, sync=False, reason="delay wv dma"
                    )
                    add_dep_helper(
                        d2.ins, anchor.ins, sync=False, reason="delay rlk dma"
                    )
                j = i - 2
                if 0 <= j < NT:
                    emit_rank1(j)
                    emit_tanh_sig(j)
                j = i - 3
                if 0 <= j < NT:
                    emit_wpath(j)
                    emit_ut(j)
                if i == NT - 1:
                    DVE.tensor_reduce(
                        uta_sb[:],
                        utp_sb[:, 0 : 12 * DC].rearrange(
                            "p (i c) -> p c i", c=DC
                        ),
                        axis=AX.X,
                        op=OP.add,
                    )

            # Tail: S = sum exp + 1e-9, A = (ut @ WvT) / S, A_lk = R * A.
            DVE.tensor_reduce(S_sb[:], sxp_sb[0:1, :], axis=AX.X, op=OP.add)
            DVE.tensor_scalar(
                out=S_sb[:], in0=S_sb[:], scalar1=1e-9, scalar2=None, op0=OP.add
            )
            DVE.reciprocal(inv_sb[:], S_sb[:])
            DVE.tensor_reduce(
                ut_sb[:],
                utp_sb[:, 12 * DC :].rearrange("p (i c) -> p c i", c=DC),
                axis=AX.X,
                op=OP.add,
            )
            DVE.tensor_tensor(out=ut_sb[:], in0=ut_sb[:], in1=uta_sb[:], op=OP.add)
            ACT.activation(utr_sb[:], ut_sb[:], AF.Copy)
            nc.gpsimd.partition_broadcast(invb_sb[:], inv_sb[0:1, 0:1])
            Aps = [kps.tile([1, 512], F32, tag="k", name=f"Aps{h}") for h in range(H)]
            for h in range(H):
                for c in range(DC):
                    PE.matmul(
                        Aps[h][0:1, :],
                        utr_sb[:, c : c + 1],
                        wv_sb[:, c, h * 512 : (h + 1) * 512],
                        start=(c == 0),
                        stop=(c == DC - 1),
                    )
            for h in range(H):
                DVE.tensor_scalar(
                    out=Af_sb[0:1, h * 512 : (h + 1) * 512],
                    in0=Aps[h][0:1, :],
                    scalar1=inv_sb[0:1, 0:1],
                    scalar2=None,
                    op0=OP.mult,
                )
                ACT.activation(
                    A_sb[0:1, h * 512 : (h + 1) * 512], Aps[h][0:1, :], AF.Copy
                )
            nc.sync.dma_start(A_out[:], Af_sb[:])
            Abps = [kps.tile([P, 512], F32, tag="k", name=f"Abps{h}") for h in range(H)]
            for h in range(H):
                PE.matmul(
                    Abps[h][:, :],
                    onesr,
                    A_sb[0:1, h * 512 : (h + 1) * 512],
                    start=True,
                    stop=True,
                )
                DVE.scalar_tensor_tensor(
                    out=alk_sb[:, h * 512 : (h + 1) * 512],
                    in0=Abps[h][:, :],
                    scalar=invb_sb[:, 0:1],
                    in1=rlk_sb[:, h * 512 : (h + 1) * 512],
                    op0=OP.mult,
                    op1=OP.mult,
                )
            nc.scalar.dma_start(Alk_out[:], alk_sb[:])

    nc.compile()
    return nc


def _get_nc():
    if "nc" not in _CACHE:
        _CACHE["nc"] = _build_nc()
    return _CACHE["nc"]


def _stage_inputs(e_i_t_minus_1, e_j_t, imp_j_t, R_lk, W_q, W_k, W_v, omega):
    """Host-side layout staging: shard over B, d-major transposes for the PE."""
    f32 = np.float32

    # W*.T tiled [P, DC, D]: w[p, c, e] = W[e, c*128+p]
    def wstage(W):
        return np.ascontiguousarray(
            W.T.reshape(DC, P, D).transpose(1, 0, 2), dtype=f32
        )

    import ml_dtypes

    bf16 = ml_dtypes.bfloat16
    wk = wstage(W_k).astype(bf16)
    wv = wstage(W_v)
    qrow = (e_i_t_minus_1.astype(np.float64) @ W_q.astype(np.float64).T).astype(f32)
    planes = np.concatenate(
        [np.eye(P, dtype=f32), np.broadcast_to(omega[None, :], (P, D))], axis=1
    )
    planes = np.ascontiguousarray(planes)
    rlk = np.ascontiguousarray(R_lk, dtype=f32)
    ones = np.ones((1, P), dtype=f32)
    maps = []
    for b in range(B):
        # eTs[g, p, j_in_group, c, j] = e_j[b, (4g+jg)*128+j, c*128+p]
        ets = np.ascontiguousarray(
            e_j_t[b].reshape(NG, EG, P, DC, P).transpose(0, 4, 1, 3, 2)
        ).astype(bf16)
        rows = np.concatenate(
            [qrow[b : b + 1], imp_j_t[b][None, :].astype(f32), ones], axis=1
        )
        maps.append(
            {
                "eTs": ets,
                "wkT": wk,
                "wvT": wv,
                "rowsp": np.ascontiguousarray(rows),
                "planes": planes,
                "rlk": rlk,
            }
        )
    return maps


def _run_pjrt_staged(nc, in_maps):
    """Like bass2jax.run_bass_via_pjrt, but device_put + block the inputs
    BEFORE executing so the NEFF's DMA doesn't contend with PJRT's input
    uploads (which otherwise stream in concurrently and halve HBM BW)."""
    import jax
    import concourse.mybir as mybir
    from jax.sharding import Mesh, NamedSharding, PartitionSpec
    from jax.experimental.shard_map import shard_map
    from concourse import bass2jax

    bass2jax.install_neuronx_cc_hook()
    n_cores = len(in_maps)
    partition_name = nc.partition_id_tensor.name if nc.partition_id_tensor else None
    in_names, out_names, out_avals, zero_outs = [], [], [], []
    for alloc in nc.m.functions[0].allocations:
        if not isinstance(alloc, mybir.MemoryLocationSet):
            continue
        name = alloc.memorylocations[0].name
        if alloc.kind == "ExternalInput":
            if name != partition_name:
                in_names.append(name)
        elif alloc.kind == "ExternalOutput":
            shape = tuple(alloc.tensor_shape)
            dtype = mybir.dt.np(alloc.dtype)
            out_names.append(name)
            out_avals.append(jax.core.ShapedArray(shape, dtype))
            zero_outs.append(np.zeros(shape, dtype))
    n_params = len(in_names)
    n_outs = len(out_avals)
    in_names = in_names + out_names
    if partition_name is not None:
        in_names.append(partition_name)

    def _body(*args):
        operands = list(args)
        if partition_name is not None:
            operands.append(bass2jax.partition_id_tensor())
        outs = bass2jax._bass_exec_p.bind(
            *operands,
            out_avals=tuple(out_avals),
            in_names=tuple(in_names),
            out_names=tuple(out_names),
            lowering_input_output_aliases=(),
            sim_require_finite=True,
            sim_require_nnan=True,
            nc=nc,
        )
        return tuple(outs)

    devices = jax.devices()[:n_cores]
    mesh = Mesh(np.asarray(devices), ("core",))
    spec = PartitionSpec("core")
    sharding = NamedSharding(mesh, spec)
    donate = tuple(range(n_params, n_params + n_outs))
    sharded = jax.jit(
        shard_map(
            _body,
            mesh=mesh,
            in_specs=(spec,) * (n_params + n_outs),
            out_specs=(spec,) * n_outs,
            check_rep=False,
        ),
        donate_argnums=donate,
        keep_unused=True,
    )
    concat_in = [
        jax.device_put(
            np.concatenate(
                [np.asarray(in_maps[c][nm]) for c in range(n_cores)], axis=0
            ),
            sharding,
        )
        for nm in in_names[:n_params]
    ]
    import jax.numpy as jnp

    zeros_fn = jax.jit(
        lambda: tuple(
            jnp.zeros((n_cores * z.shape[0], *z.shape[1:]), z.dtype)
            for z in zero_outs
        ),
        out_shardings=(sharding,) * n_outs,
    )
    concat_zeros = list(zeros_fn())
    jax.block_until_ready(concat_in)
    jax.block_until_ready(concat_zeros)
    out_arrs = sharded(*concat_in, *concat_zeros)
    return [
        {
            nm: np.asarray(out_arrs[i]).reshape(n_cores, *out_avals[i].shape)[c]
            for i, nm in enumerate(out_names)
        }
        for c in range(n_cores)
    ]


def kernel(e_i_t_minus_1, e_j_t, imp_j_t, R_lk, W_q, W_k, W_v, omega):
    nc = _get_nc()
    in_maps = _stage_inputs(
        e_i_t_minus_1, e_j_t, imp_j_t, R_lk, W_q, W_k, W_v, omega
    )
    results = _run_pjrt_staged(nc, in_maps)
    A = np.stack([results[b]["A_out"][0] for b in range(B)]).astype(np.float32)
    A_lk = np.stack([results[b]["Alk_out"] for b in range(B)]).astype(np.float32)
    return (A, A_lk, A)


# revision 24
# speedup vs baseline: 1.0554x; 1.0554x over previous
# Bass/Trainium2 SPMD kernel for nn_AdaptiveIncrementalAttention.
#
# Math (per batch b):
#   q = W_q @ e_i[b]                                     [D]   (host-staged)
#   k[n, e] = sum_d e_j[b, n, d] * W_k[e, d]             [N, D]
#   sigma[n] = sum_e omega[e] * tanh(q[e] + k[n, e])     [N]
#   a[n] = imp[n] * exp(sigma[n]) / (sum_n exp(sigma) + 1e-9)
#   u[d] = sum_n a[n] * e_j[b, n, d]                     [D]   (weighted sum
#   A[e] = sum_d u[d] * W_v[e, d]                        [D]    commutes with W_v)
#   A_lk[l, d] = A[d] * R_lk[l, d]                       [L, D] (outer broadcast)
#   returns (A, A_lk, A)  -- A_l is identical to A in the source module.
#
# Sharding: data-parallel over batch B=8 across the 8 NeuronCores; weights
# replicated, no collectives. Host pre-transposes W_k/W_v and the e_j shards
# into the d-major layouts the PE array contracts over (layout staging only).
# The N x D projection runs as float32r (full-rate fp32 PE mode); softmax /
# weighted-sum pipeline overlaps it tile by tile.
import sys
import os

sys.path.insert(0, "/opt/trn_rl_repo")
import numpy as np

B, N, D, L = 8, 2048, 1024, 128
P = 128
NT = N // P  # 16 n-tiles
DC = D // P  # 8 d-chunks
H = D // 512  # 2 psum halves per row block
EG = 4  # eT tiles per DMA group
NG = NT // EG  # 4 groups
ROWS = D + N + P  # packed row-vector input: qrow | imp | ones

_CACHE = {}


def _build_nc():
    import concourse.mybir as mybir
    import concourse.tile as tile
    from concourse import bacc

    F32 = mybir.dt.float32
    F32R = mybir.dt.float32r
    BF16 = mybir.dt.bfloat16
    AF = mybir.ActivationFunctionType
    OP = mybir.AluOpType
    AX = mybir.AxisListType

    nc = bacc.Bacc(None, target_bir_lowering=False, debug=False)

    # Per-core inputs (host-staged layouts; see _stage_inputs below).
    eTs = nc.declare_dram_parameter("eTs", [NG, P, EG, DC, P], BF16, isOutput=False)
    wkT = nc.declare_dram_parameter("wkT", [P, DC, D], BF16, isOutput=False)
    wvT = nc.declare_dram_parameter("wvT", [P, DC, D], F32R, isOutput=False)
    rowsp = nc.declare_dram_parameter("rowsp", [1, ROWS], F32R, isOutput=False)
    planes = nc.declare_dram_parameter("planes", [P, P + D], F32, isOutput=False)
    rlk = nc.declare_dram_parameter("rlk", [L, D], F32, isOutput=False)
    A_out = nc.declare_dram_parameter("A_out", [1, D], F32, isOutput=True)
    Alk_out = nc.declare_dram_parameter("Alk_out", [L, D], F32, isOutput=True)

    with tile.TileContext(nc) as tc:
        with (
            tc.tile_pool(name="per", bufs=1) as per,
            tc.tile_pool(name="et", bufs=3) as etp,
            tc.tile_pool(name="tt", bufs=2) as ttp,
            tc.tile_pool(name="scr", bufs=2) as scr,
            tc.tile_pool(name="kps", bufs=6, space="PSUM") as kps,
            tc.tile_pool(name="wps", bufs=2, space="PSUM") as wps,
        ):
            PE, ACT, DVE = nc.tensor, nc.scalar, nc.vector

            # Persistent SBUF residents.
            wk_sb = per.tile([P, DC, D], BF16, tag="wk")
            wv_sb = per.tile([P, DC, D], F32R, tag="wv")
            rows_sb = per.tile([1, ROWS], F32R, tag="rows")
            planes_sb = per.tile([P, P + D], F32, tag="planes")
            rlk_sb = per.tile([L, D], F32, tag="rlk")
            sig_sb = per.tile([P, NT], F32, tag="sig")
            exr_sb = per.tile([1, N], F32, tag="exr")
            wrf_sb = per.tile([1, N], BF16, tag="wrf")
            sxp_sb = per.tile([1, NT], F32, tag="sxp")
            utp_sb = per.tile([P, DC * NT], F32, tag="utp")
            ut_sb = per.tile([P, DC], F32, tag="ut")
            uta_sb = per.tile([P, DC], F32, tag="uta")
            utr_sb = per.tile([P, DC], F32R, tag="utr")
            A_sb = per.tile([1, D], F32R, tag="Asb")
            Af_sb = per.tile([1, D], F32, tag="Afsb")
            Ab_sb = per.tile([P, D], F32, tag="Absb")
            alk_sb = per.tile([L, D], F32, tag="alk")
            S_sb = per.tile([1, 1], F32, tag="S")
            inv_sb = per.tile([1, 1], F32, tag="inv")
            invb_sb = per.tile([P, 1], F32, tag="invb")
            onesbf_sb = per.tile([1, P], BF16, tag="onesbf")

            qrow = rows_sb[0:1, 0:D]
            impr = rows_sb[0:1, D : D + N].bitcast(F32)
            onesr = rows_sb[0:1, D + N : D + N + P]
            idnt = planes_sb[:, 0:P]
            omgb = planes_sb[:, P : P + D]

            # Head DMAs: small packs + first eT group + W_k halves, ordered so
            # tile 0's operands land first. eT groups stream on the scalar
            # (ACT) HWDGE so descriptor generation runs parallel to sync's.
            nc.sync.dma_start(rows_sb[:], rowsp[:])
            et_s = {}
            et_g = {}
            for i in range(EG):
                et_s[i] = etp.tile([P, DC, P], BF16, tag="et1", name=f"ets{i}", bufs=4)
            nc.sync.dma_start(et_s[0][:], eTs[0][:, 0, :, :])
            nc.sync.dma_start(wk_sb[:, 0:4, :], wkT[:, 0:4, :])
            nc.sync.dma_start(wk_sb[:, 4:8, :], wkT[:, 4:8, :])
            nc.scalar.dma_start(et_s[1][:], eTs[0][:, 1, :, :])
            nc.scalar.dma_start(et_s[2][:], eTs[0][:, 2, :, :])
            nc.scalar.dma_start(et_s[3][:], eTs[0][:, 3, :, :])
            nc.sync.dma_start(planes_sb[:], planes[:])

            ACT.activation(onesbf_sb[0:1, :], onesr.bitcast(F32), AF.Copy)

            def et_tile(i):
                if i < EG:
                    return et_s[i][:, :, :]
                return et_g[i // EG][:, i % EG, :, :]

            kp_tiles = {}
            wb_tiles = {}
            pb_insts = {}

            def emit_dmms(i):
                kph = [kps.tile([P, 512], F32, tag="k", name=f"kp{i}_{h}") for h in range(H)]
                kp_tiles[i] = kph
                et = et_tile(i)
                for c in range(DC):
                    for h in range(H):
                        PE.matmul(
                            kph[h][:, :],
                            et[:, c, :],
                            wk_sb[:, c, h * 512 : (h + 1) * 512],
                            start=(c == 0),
                            stop=False,
                        )

            def emit_rank1(i):
                # Close the accumulation with k += ones ⊗ q_row (the q bias).
                for h in range(H):
                    PE.matmul(
                        kp_tiles[i][h][:, :],
                        onesr,
                        qrow[0:1, h * 512 : (h + 1) * 512],
                        start=False,
                        stop=True,
                    )

            def emit_tanh_sig(i):
                t = ttp.tile([P, D], F32, tag="t")
                for h in range(H):
                    ACT.activation(t[:, h * 512 : (h + 1) * 512], kp_tiles[i][h][:, :], AF.Tanh)
                s = scr.tile([P, D], BF16, tag="sbf")
                DVE.scalar_tensor_tensor(
                    out=s[:],
                    in0=t[:],
                    scalar=1.0,
                    in1=omgb,
                    op0=OP.mult,
                    op1=OP.mult,
                    accum_out=sig_sb[:, i : i + 1],
                )

            def emit_wpath(i):
                # sigma col -> row, exp (with per-tile sum), w = imp * exp,
                # then broadcast w across partitions for the u-accumulation.
                srp = wps.tile([1, P], F32, tag="w")
                PE.transpose(srp[:], sig_sb[:, i : i + 1], idnt)
                ACT.activation(
                    exr_sb[0:1, i * P : (i + 1) * P],
                    srp[0:1, :],
                    AF.Exp,
                    accum_out=sxp_sb[0:1, i : i + 1],
                )
                DVE.tensor_tensor(
                    out=wrf_sb[0:1, i * P : (i + 1) * P],
                    in0=exr_sb[0:1, i * P : (i + 1) * P],
                    in1=impr[0:1, i * P : (i + 1) * P],
                    op=OP.mult,
                )
                wrf8 = scr.tile([1, DC * P], BF16, tag="wrf8")
                ACT.activation(
                    wrf8[0:1, :],
                    wrf_sb[0:1, i * P : (i + 1) * P]
                    .rearrange("o (x j) -> o x j", x=1)
                    .broadcast_to([1, DC, P]),
                    AF.Copy,
                )
                wb = scr.tile([P, DC * P], BF16, tag="wb")
                wb_tiles[i] = wb
                pb = nc.gpsimd.partition_broadcast(wb[:], wrf8[0:1, :])
                pb_insts[i] = pb

            def emit_ut(i):
                # us = eT_tile * w8 (flat 2-D bf16 for the fast DVE mode),
                # then per-chunk row sums into utp[:, i*DC:(i+1)*DC].
                us = scr.tile([P, DC * P], BF16, tag="us")
                et = et_tile(i).rearrange("p c j -> p (c j)")
                DVE.tensor_tensor(out=us[:], in0=et, in1=wb_tiles[i][:], op=OP.mult)
                DVE.tensor_reduce(
                    utp_sb[:, i * DC : (i + 1) * DC],
                    us[:].rearrange("p (c j) -> p c j", c=DC),
                    axis=AX.X,
                    op=OP.add,
                )

            # Software-pipelined main loop: rank1/tanh lag 2, w/u lag 3 so the
            # PE stream never waits on the ACT/DVE chain of the same tile.
            for i in range(NT + 3):
                if i < NT:
                    emit_dmms(i)
                if i == 0:
                    et_g[1] = etp.tile([P, EG, DC, P], BF16, tag="et", name="etg1")
                    nc.scalar.dma_start(et_g[1][:], eTs[1])
                if i == 3:
                    et_g[2] = etp.tile([P, EG, DC, P], BF16, tag="et", name="etg2")
                    nc.scalar.dma_start(et_g[2][:], eTs[2])
                if i == 10:
                    et_g[3] = etp.tile([P, EG, DC, P], BF16, tag="et", name="etg3")
                    nc.scalar.dma_start(et_g[3][:], eTs[3])
                if i == 6:
                    from concourse.tile_rust import add_dep_helper

                    d1 = nc.gpsimd.dma_start(wv_sb[:], wvT[:])
                    d2 = nc.gpsimd.dma_start(rlk_sb[:], rlk[:])
                    anchor = pb_insts[2]
                    add_dep_helper(
                        d1.ins, anchor.ins, sync=False, reason="delay wv dma"
                    )
                    add_dep_helper(
                        d2.ins, anchor.ins, sync=False, reason="delay rlk dma"
                    )
                j = i - 2
                if 0 <= j < NT:
                    emit_rank1(j)
                    emit_tanh_sig(j)
                j = i - 3
                if 0 <= j < NT:
                    emit_wpath(j)
                    emit_ut(j)
                if i == NT - 1:
                    DVE.tensor_reduce(
                        uta_sb[:],
                        utp_sb[:, 0 : 12 * DC].rearrange(
                            "p (i c) -> p c i", c=DC
                        ),
                        axis=AX.X,
                        op=OP.add,
                    )

            # Tail: S = sum exp + 1e-9, A = (ut @ WvT) / S, A_lk = R * A.
            DVE.tensor_reduce(S_sb[:], sxp_sb[0:1, :], axis=AX.X, op=OP.add)
            DVE.tensor_scalar(
                out=S_sb[:], in0=S_sb[:], scalar1=1e-9, scalar2=None, op0=OP.add
            )
            DVE.reciprocal(inv_sb[:], S_sb[:])
            DVE.tensor_reduce(
                ut_sb[:],
                utp_sb[:, 12 * DC :].rearrange("p (i c) -> p c i", c=DC),
                axis=AX.X,
                op=OP.add,
            )
            DVE.tensor_tensor(out=ut_sb[:], in0=ut_sb[:], in1=uta_sb[:], op=OP.add)
            ACT.activation(utr_sb[:], ut_sb[:], AF.Copy)
            nc.gpsimd.partition_broadcast(invb_sb[:], inv_sb[0:1, 0:1])
            Aps = [kps.tile([1, 512], F32, tag="k", name=f"Aps{h}") for h in range(H)]
            for h in range(H):
                for c in range(DC):
                    PE.matmul(
                        Aps[h][0:1, :],
                        utr_sb[:, c : c + 1],
                        wv_sb[:, c, h * 512 : (h + 1) * 512],
                        start=(c == 0),
                        stop=(c == DC - 1),
                    )
            for h in range(H):
                DVE.tensor_scalar(
                    out=Af_sb[0:1, h * 512 : (h + 1) * 512],
                    in0=Aps[h][0:1, :],
                    scalar1=inv_sb[0:1, 0:1],
                    scalar2=None,
                    op0=OP.mult,
                )
                ACT.activation(
                    A_sb[0:1, h * 512 : (h + 1) * 512], Aps[h][0:1, :], AF.Copy
                )
            nc.sync.dma_start(A_out[:], Af_sb[:])
            Abps = [kps.tile([P, 512], F32, tag="k", name=f"Abps{h}") for h in range(H)]
            for h in range(H):
                PE.matmul(
                    Abps[h][:, :],
                    onesr,
                    A_sb[0:1, h * 512 : (h + 1) * 512],
                    start=True,
                    stop=True,
                )
                DVE.scalar_tensor_tensor(
                    out=alk_sb[:, h * 512 : (h + 1) * 512],
                    in0=Abps[h][:, :],
                    scalar=invb_sb[:, 0:1],
                    in1=rlk_sb[:, h * 512 : (h + 1) * 512],
                    op0=OP.mult,
                    op1=OP.mult,
                )
            nc.scalar.dma_start(Alk_out[:], alk_sb[:])

    nc.compile()
    return nc


def _get_nc():
    if "nc" not in _CACHE:
        _CACHE["nc"] = _build_nc()
    return _CACHE["nc"]


def _stage_inputs(e_i_t_minus_1, e_j_t, imp_j_t, R_lk, W_q, W_k, W_v, omega):
    """Host-side layout staging: shard over B, d-major transposes for the PE."""
    f32 = np.float32

    # W*.T tiled [P, DC, D]: w[p, c, e] = W[e, c*128+p]
    def wstage(W):
        return np.ascontiguousarray(
            W.T.reshape(DC, P, D).transpose(1, 0, 2), dtype=f32
        )

    import ml_dtypes

    bf16 = ml_dtypes.bfloat16
    wk = wstage(W_k).astype(bf16)
    wv = wstage(W_v)
    qrow = (e_i_t_minus_1.astype(np.float64) @ W_q.astype(np.float64).T).astype(f32)
    planes = np.concatenate(
        [np.eye(P, dtype=f32), np.broadcast_to(omega[None, :], (P, D))], axis=1
    )
    planes = np.ascontiguousarray(planes)
    rlk = np.ascontiguousarray(R_lk, dtype=f32)
    ones = np.ones((1, P), dtype=f32)
    maps = []
    for b in range(B):
        # eTs[g, p, j_in_group, c, j] = e_j[b, (4g+jg)*128+j, c*128+p]
        ets = np.ascontiguousarray(
            e_j_t[b].reshape(NG, EG, P, DC, P).transpose(0, 4, 1, 3, 2)
        ).astype(bf16)
        rows = np.concatenate(
            [qrow[b : b + 1], imp_j_t[b][None, :].astype(f32), ones], axis=1
        )
        maps.append(
            {
                "eTs": ets,
                "wkT": wk,
                "wvT": wv,
                "rowsp": np.ascontiguousarray(rows),
                "planes": planes,
                "rlk": rlk,
            }
        )
    return maps


def _run_pjrt_staged(nc, in_maps):
    """Like bass2jax.run_bass_via_pjrt, but device_put + block the inputs
    BEFORE executing so the NEFF's DMA doesn't contend with PJRT's input
    uploads (which otherwise stream in concurrently and halve HBM BW)."""
    import jax
    import concourse.mybir as mybir
    from jax.sharding import Mesh, NamedSharding, PartitionSpec
    from jax.experimental.shard_map import shard_map
    from concourse import bass2jax

    bass2jax.install_neuronx_cc_hook()
    n_cores = len(in_maps)
    partition_name = nc.partition_id_tensor.name if nc.partition_id_tensor else None
    in_names, out_names, out_avals, zero_outs = [], [], [], []
    for alloc in nc.m.functions[0].allocations:
        if not isinstance(alloc, mybir.MemoryLocationSet):
            continue
        name = alloc.memorylocations[0].name
        if alloc.kind == "ExternalInput":
            if name != partition_name:
                in_names.append(name)
        elif alloc.kind == "ExternalOutput":
            shape = tuple(alloc.tensor_shape)
            dtype = mybir.dt.np(alloc.dtype)
            out_names.append(name)
            out_avals.append(jax.core.ShapedArray(shape, dtype))
            zero_outs.append(np.zeros(shape, dtype))
    n_params = len(in_names)
    n_outs = len(out_avals)
    in_names = in_names + out_names
    if partition_name is not None:
        in_names.append(partition_name)

    def _body(*args):
        operands = list(args)
        if partition_name is not None:
            operands.append(bass2jax.partition_id_tensor())
        outs = bass2jax._bass_exec_p.bind(
            *operands,
            out_avals=tuple(out_avals),
            in_names=tuple(in_names),
            out_names=tuple(out_names),
            lowering_input_output_aliases=(),
            sim_require_finite=True,
            sim_require_nnan=True,
            nc=nc,
        )
        return tuple(outs)

    devices = jax.devices()[:n_cores]
    mesh = Mesh(np.asarray(devices), ("core",))
    spec = PartitionSpec("core")
    sharding = NamedSharding(mesh, spec)
    donate = tuple(range(n_params, n_params + n_outs))
    sharded = jax.jit(
        shard_map(
            _body,
            mesh=mesh,
            in_specs=(spec,) * (n_params + n_outs),
            out_specs=(spec,) * n_outs,
            check_rep=False,
        ),
        donate_argnums=donate,
        keep_unused=True,
    )
    concat_in = [
        jax.device_put(
            np.concatenate(
                [np.asarray(in_maps[c][nm]) for c in range(n_cores)], axis=0
            ),
            sharding,
        )
        for nm in in_names[:n_params]
    ]
    import jax.numpy as jnp

    zeros_fn = jax.jit(
        lambda: tuple(
            jnp.zeros((n_cores * z.shape[0], *z.shape[1:]), z.dtype)
            for z in zero_outs
        ),
        out_shardings=(sharding,) * n_outs,
    )
    concat_zeros = list(zeros_fn())
    jax.block_until_ready(concat_in)
    jax.block_until_ready(concat_zeros)
    out_arrs = sharded(*concat_in, *concat_zeros)
    return [
        {
            nm: np.asarray(out_arrs[i]).reshape(n_cores, *out_avals[i].shape)[c]
            for i, nm in enumerate(out_names)
        }
        for c in range(n_cores)
    ]


def kernel(e_i_t_minus_1, e_j_t, imp_j_t, R_lk, W_q, W_k, W_v, omega):
    nc = _get_nc()
    in_maps = _stage_inputs(
        e_i_t_minus_1, e_j_t, imp_j_t, R_lk, W_q, W_k, W_v, omega
    )
    results = _run_pjrt_staged(nc, in_maps)
    A = np.stack([results[b]["A_out"][0] for b in range(B)]).astype(np.float32)
    A_lk = np.stack([results[b]["Alk_out"] for b in range(B)]).astype(np.float32)
    return (A, A_lk, A)


# revision 26
# speedup vs baseline: 1.1683x; 1.1070x over previous
# Bass/Trainium2 SPMD kernel for nn_AdaptiveIncrementalAttention.
#
# Math (per batch b):
#   q = W_q @ e_i[b]                                     [D]   (host-staged)
#   k[n, e] = sum_d e_j[b, n, d] * W_k[e, d]             [N, D]
#   sigma[n] = sum_e omega[e] * tanh(q[e] + k[n, e])     [N]
#   a[n] = imp[n] * exp(sigma[n]) / (sum_n exp(sigma) + 1e-9)
#   u[d] = sum_n a[n] * e_j[b, n, d]                     [D]   (weighted sum
#   A[e] = sum_d u[d] * W_v[e, d]                        [D]    commutes with W_v)
#   A_lk[l, d] = A[d] * R_lk[l, d]                       [L, D] (outer broadcast)
#   returns (A, A_lk, A)  -- A_l is identical to A in the source module.
#
# Sharding: data-parallel over batch B=8 across the 8 NeuronCores; weights
# replicated, no collectives. Host pre-transposes W_k/W_v and the e_j shards
# into the d-major layouts the PE array contracts over (layout staging only).
# The N x D projection runs as float32r (full-rate fp32 PE mode); softmax /
# weighted-sum pipeline overlaps it tile by tile.
import sys
import os

sys.path.insert(0, "/opt/trn_rl_repo")
import numpy as np

B, N, D, L = 8, 2048, 1024, 128
P = 128
NT = N // P  # 16 n-tiles
DC = D // P  # 8 d-chunks
H = D // 512  # 2 psum halves per row block
EG = 4  # eT tiles per DMA group
NG = NT // EG  # 4 groups
ROWS = D + N + P  # packed row-vector input: qrow | imp | ones

_CACHE = {}


def _build_nc():
    import concourse.mybir as mybir
    import concourse.tile as tile
    from concourse import bacc

    F32 = mybir.dt.float32
    F32R = mybir.dt.float32r
    BF16 = mybir.dt.bfloat16
    AF = mybir.ActivationFunctionType
    OP = mybir.AluOpType
    AX = mybir.AxisListType

    nc = bacc.Bacc(None, target_bir_lowering=False, debug=False)

    # Per-core inputs (host-staged layouts; see _stage_inputs below).
    eTs = nc.declare_dram_parameter("eTs", [NG, P, EG, DC, P], BF16, isOutput=False)
    wkT = nc.declare_dram_parameter("wkT", [P, DC, D], BF16, isOutput=False)
    wvT = nc.declare_dram_parameter("wvT", [P, DC, D], F32R, isOutput=False)
    rowsp = nc.declare_dram_parameter("rowsp", [1, ROWS], F32R, isOutput=False)
    planes = nc.declare_dram_parameter("planes", [P, P + D], F32, isOutput=False)
    rlk = nc.declare_dram_parameter("rlk", [L, D], F32, isOutput=False)
    A_out = nc.declare_dram_parameter("A_out", [1, D], F32, isOutput=True)
    Alk_out = nc.declare_dram_parameter("Alk_out", [L, D], F32, isOutput=True)

    with tile.TileContext(nc) as tc:
        with (
            tc.tile_pool(name="per", bufs=1) as per,
            tc.tile_pool(name="et", bufs=3) as etp,
            tc.tile_pool(name="tt", bufs=2) as ttp,
            tc.tile_pool(name="scr", bufs=2) as scr,
            tc.tile_pool(name="kps", bufs=7, space="PSUM") as kps,
            tc.tile_pool(name="wps", bufs=1, space="PSUM") as wps,
        ):
            PE, ACT, DVE = nc.tensor, nc.scalar, nc.vector

            # Persistent SBUF residents.
            wk_sb = per.tile([P, DC, D], BF16, tag="wk")
            wv_sb = per.tile([P, DC, D], F32R, tag="wv")
            rows_sb = per.tile([1, ROWS], F32R, tag="rows")
            planes_sb = per.tile([P, P + D], F32, tag="planes")
            rlk_sb = per.tile([L, D], F32, tag="rlk")
            sig_sb = per.tile([P, NT], F32, tag="sig")
            exr_sb = per.tile([1, N], F32, tag="exr")
            wrf_sb = per.tile([1, N], BF16, tag="wrf")
            sxp_sb = per.tile([1, NT], F32, tag="sxp")
            utp_sb = per.tile([P, DC * NT], F32, tag="utp")
            ut_sb = per.tile([P, DC], F32, tag="ut")
            uta_sb = per.tile([P, DC], F32, tag="uta")
            utr_sb = per.tile([P, DC], F32R, tag="utr")
            A_sb = per.tile([1, D], F32R, tag="Asb")
            Af_sb = per.tile([1, D], F32, tag="Afsb")
            Ab_sb = per.tile([P, D], F32, tag="Absb")
            alk_sb = per.tile([L, D], F32, tag="alk")
            S_sb = per.tile([1, 1], F32, tag="S")
            inv_sb = per.tile([1, 1], F32, tag="inv")
            invb_sb = per.tile([P, 1], F32, tag="invb")
            onesbf_sb = per.tile([1, P], BF16, tag="onesbf")

            qrow = rows_sb[0:1, 0:D]
            impr = rows_sb[0:1, D : D + N].bitcast(F32)
            onesr = rows_sb[0:1, D + N : D + N + P]
            idnt = planes_sb[:, 0:P]
            omgb = planes_sb[:, P : P + D]

            # Head DMAs: small packs + first eT group + W_k halves, ordered so
            # tile 0's operands land first. eT groups stream on the scalar
            # (ACT) HWDGE so descriptor generation runs parallel to sync's.
            nc.sync.dma_start(rows_sb[:], rowsp[:])
            et_s = {}
            et_g = {}
            for i in range(EG):
                et_s[i] = etp.tile([P, DC, P], BF16, tag="et1", name=f"ets{i}", bufs=4)
            nc.sync.dma_start(et_s[0][:], eTs[0][:, 0, :, :])
            nc.sync.dma_start(wk_sb[:, 0:4, :], wkT[:, 0:4, :])
            nc.scalar.dma_start(et_s[1][:], eTs[0][:, 1, :, :])
            nc.sync.dma_start(wk_sb[:, 4:8, :], wkT[:, 4:8, :])

            ACT.activation(onesbf_sb[0:1, :], onesr.bitcast(F32), AF.Copy)

            def et_tile(i):
                if i < EG:
                    return et_s[i][:, :, :]
                return et_g[i // EG][:, i % EG, :, :]

            kp_tiles = {}
            wb_tiles = {}
            pb_insts = {}

            def emit_dmms(i):
                kph = [kps.tile([P, 512], F32, tag="k", name=f"kp{i}_{h}") for h in range(H)]
                kp_tiles[i] = kph
                et = et_tile(i)
                for c in range(DC):
                    for h in range(H):
                        PE.matmul(
                            kph[h][:, :],
                            et[:, c, :],
                            wk_sb[:, c, h * 512 : (h + 1) * 512],
                            start=(c == 0),
                            stop=False,
                        )

            def emit_rank1(i):
                # Close the accumulation with k += ones ⊗ q_row (the q bias).
                for h in range(H):
                    PE.matmul(
                        kp_tiles[i][h][:, :],
                        onesr,
                        qrow[0:1, h * 512 : (h + 1) * 512],
                        start=False,
                        stop=True,
                    )

            def emit_tanh_sig(i):
                t = ttp.tile([P, D], F32, tag="t")
                for h in range(H):
                    ACT.activation(t[:, h * 512 : (h + 1) * 512], kp_tiles[i][h][:, :], AF.Tanh)
                s = scr.tile([P, D], BF16, tag="sbf")
                DVE.scalar_tensor_tensor(
                    out=s[:],
                    in0=t[:],
                    scalar=1.0,
                    in1=omgb,
                    op0=OP.mult,
                    op1=OP.mult,
                    accum_out=sig_sb[:, i : i + 1],
                )

            def emit_wpath(i):
                # sigma col -> row, exp (with per-tile sum), w = imp * exp,
                # then broadcast w across partitions for the u-accumulation.
                srp = wps.tile([1, P], F32, tag="w")
                PE.transpose(srp[:], sig_sb[:, i : i + 1], idnt)
                ACT.activation(
                    exr_sb[0:1, i * P : (i + 1) * P],
                    srp[0:1, :],
                    AF.Exp,
                    accum_out=sxp_sb[0:1, i : i + 1],
                )
                DVE.tensor_tensor(
                    out=wrf_sb[0:1, i * P : (i + 1) * P],
                    in0=exr_sb[0:1, i * P : (i + 1) * P],
                    in1=impr[0:1, i * P : (i + 1) * P],
                    op=OP.mult,
                )
                wb = scr.tile([P, P], BF16, tag="wb")
                wb_tiles[i] = wb
                pb = nc.gpsimd.partition_broadcast(
                    wb[:], wrf_sb[0:1, i * P : (i + 1) * P]
                )
                pb_insts[i] = pb

            def emit_ut(i):
                # us = eT_tile * w (w broadcast over d-chunks), then per-chunk
                # row sums into utp[:, i*DC:(i+1)*DC].
                us = scr.tile([P, DC, P], BF16, tag="us")
                et = et_tile(i)
                wb_b = (
                    wb_tiles[i][:]
                    .rearrange("p (o j) -> p o j", o=1)
                    .broadcast_to([P, DC, P])
                )
                DVE.tensor_tensor(out=us[:], in0=et[:, :, :], in1=wb_b, op=OP.mult)
                DVE.tensor_reduce(
                    utp_sb[:, i * DC : (i + 1) * DC],
                    us[:],
                    axis=AX.X,
                    op=OP.add,
                )

            # Software-pipelined main loop: rank1/tanh lag 2, w/u lag 3 so the
            # PE stream never waits on the ACT/DVE chain of the same tile.
            for i in range(NT + 3):
                if i < NT:
                    emit_dmms(i)
                if i == 0:
                    nc.scalar.dma_start(et_s[2][:], eTs[0][:, 2, :, :])
                    nc.scalar.dma_start(et_s[3][:], eTs[0][:, 3, :, :])
                    nc.sync.dma_start(planes_sb[:], planes[:])
                    et_g[1] = etp.tile([P, EG, DC, P], BF16, tag="et", name="etg1")
                    nc.scalar.dma_start(et_g[1][:], eTs[1])
                if i == 3:
                    et_g[2] = etp.tile([P, EG, DC, P], BF16, tag="et", name="etg2")
                    nc.scalar.dma_start(et_g[2][:], eTs[2])
                if i == 10:
                    et_g[3] = etp.tile([P, EG, DC, P], BF16, tag="et", name="etg3")
                    nc.scalar.dma_start(et_g[3][:], eTs[3])
                if i == 6:
                    from concourse.tile_rust import add_dep_helper

                    d1 = nc.gpsimd.dma_start(wv_sb[:], wvT[:])
                    d2 = nc.gpsimd.dma_start(rlk_sb[:], rlk[:])
                    anchor = pb_insts[2]
                    add_dep_helper(
                        d1.ins, anchor.ins, sync=False, reason="delay wv dma"
                    )
                    add_dep_helper(
                        d2.ins, anchor.ins, sync=False, reason="delay rlk dma"
                    )
                j = i - 2
                if 0 <= j < NT:
                    emit_rank1(j)
                    emit_tanh_sig(j)
                j = i - 3
                if 0 <= j < NT:
                    emit_wpath(j)
                    emit_ut(j)
                if i == NT - 1:
                    DVE.tensor_reduce(
                        uta_sb[:],
                        utp_sb[:, 0 : 12 * DC].rearrange(
                            "p (i c) -> p c i", c=DC
                        ),
                        axis=AX.X,
                        op=OP.add,
                    )

            # Tail: S = sum exp + 1e-9, A = (ut @ WvT) / S, A_lk = R * A.
            DVE.tensor_reduce(S_sb[:], sxp_sb[0:1, :], axis=AX.X, op=OP.add)
            DVE.tensor_scalar(
                out=S_sb[:], in0=S_sb[:], scalar1=1e-9, scalar2=None, op0=OP.add
            )
            DVE.reciprocal(inv_sb[:], S_sb[:])
            DVE.tensor_reduce(
                ut_sb[:],
                utp_sb[:, 12 * DC :].rearrange("p (i c) -> p c i", c=DC),
                axis=AX.X,
                op=OP.add,
            )
            DVE.tensor_tensor(out=ut_sb[:], in0=ut_sb[:], in1=uta_sb[:], op=OP.add)
            ACT.activation(utr_sb[:], ut_sb[:], AF.Copy)
            Aps = [
                [kps.tile([1, 512], F32, tag="k", name=f"Aps{h}_{p}") for p in range(2)]
                for h in range(H)
            ]
            for h in range(H):
                for c in range(DC):
                    PE.matmul(
                        Aps[h][c % 2][0:1, :],
                        utr_sb[:, c : c + 1],
                        wv_sb[:, c, h * 512 : (h + 1) * 512],
                        start=(c < 2),
                        stop=(c >= DC - 2),
                    )
            for h in range(H):
                DVE.tensor_scalar(
                    out=Af_sb[0:1, h * 512 : (h + 1) * 512],
                    in0=Aps[h][0][0:1, :],
                    scalar1=inv_sb[0:1, 0:1],
                    scalar2=None,
                    op0=OP.mult,
                )
                DVE.scalar_tensor_tensor(
                    out=Af_sb[0:1, h * 512 : (h + 1) * 512],
                    in0=Aps[h][1][0:1, :],
                    scalar=inv_sb[0:1, 0:1],
                    in1=Af_sb[0:1, h * 512 : (h + 1) * 512],
                    op0=OP.mult,
                    op1=OP.add,
                )
                ACT.activation(
                    A_sb[0:1, h * 512 : (h + 1) * 512],
                    Af_sb[0:1, h * 512 : (h + 1) * 512],
                    AF.Copy,
                )
            nc.sync.dma_start(A_out[:], Af_sb[:])
            Abps = [kps.tile([P, 512], F32, tag="k", name=f"Abps{h}") for h in range(H)]
            for h in range(H):
                PE.matmul(
                    Abps[h][:, :],
                    onesr,
                    A_sb[0:1, h * 512 : (h + 1) * 512],
                    start=True,
                    stop=True,
                )
                DVE.tensor_tensor(
                    out=alk_sb[:, h * 512 : (h + 1) * 512],
                    in0=Abps[h][:, :],
                    in1=rlk_sb[:, h * 512 : (h + 1) * 512],
                    op=OP.mult,
                )
            nc.scalar.dma_start(Alk_out[:], alk_sb[:])

    nc.compile()
    return nc


def _get_nc():
    if "nc" not in _CACHE:
        _CACHE["nc"] = _build_nc()
    return _CACHE["nc"]


def _stage_inputs(e_i_t_minus_1, e_j_t, imp_j_t, R_lk, W_q, W_k, W_v, omega):
    """Host-side layout staging: shard over B, d-major transposes for the PE."""
    f32 = np.float32

    # W*.T tiled [P, DC, D]: w[p, c, e] = W[e, c*128+p]
    def wstage(W):
        return np.ascontiguousarray(
            W.T.reshape(DC, P, D).transpose(1, 0, 2), dtype=f32
        )

    import ml_dtypes

    bf16 = ml_dtypes.bfloat16
    wk = wstage(W_k).astype(bf16)
    wv = wstage(W_v)
    qrow = (e_i_t_minus_1.astype(np.float64) @ W_q.astype(np.float64).T).astype(f32)
    planes = np.concatenate(
        [np.eye(P, dtype=f32), np.broadcast_to(omega[None, :], (P, D))], axis=1
    )
    planes = np.ascontiguousarray(planes)
    rlk = np.ascontiguousarray(R_lk, dtype=f32)
    ones = np.ones((1, P), dtype=f32)
    maps = []
    for b in range(B):
        # eTs[g, p, j_in_group, c, j] = e_j[b, (4g+jg)*128+j, c*128+p]
        ets = np.ascontiguousarray(
            e_j_t[b].reshape(NG, EG, P, DC, P).transpose(0, 4, 1, 3, 2)
        ).astype(bf16)
        rows = np.concatenate(
            [qrow[b : b + 1], imp_j_t[b][None, :].astype(f32), ones], axis=1
        )
        maps.append(
            {
                "eTs": ets,
                "wkT": wk,
                "wvT": wv,
                "rowsp": np.ascontiguousarray(rows),
                "planes": planes,
                "rlk": rlk,
            }
        )
    return maps


def _run_pjrt_staged(nc, in_maps):
    """Like bass2jax.run_bass_via_pjrt, but device_put + block the inputs
    BEFORE executing so the NEFF's DMA doesn't contend with PJRT's input
    uploads (which otherwise stream in concurrently and halve HBM BW)."""
    import jax
    import concourse.mybir as mybir
    from jax.sharding import Mesh, NamedSharding, PartitionSpec
    from jax.experimental.shard_map import shard_map
    from concourse import bass2jax

    bass2jax.install_neuronx_cc_hook()
    n_cores = len(in_maps)
    partition_name = nc.partition_id_tensor.name if nc.partition_id_tensor else None
    in_names, out_names, out_avals, zero_outs = [], [], [], []
    for alloc in nc.m.functions[0].allocations:
        if not isinstance(alloc, mybir.MemoryLocationSet):
            continue
        name = alloc.memorylocations[0].name
        if alloc.kind == "ExternalInput":
            if name != partition_name:
                in_names.append(name)
        elif alloc.kind == "ExternalOutput":
            shape = tuple(alloc.tensor_shape)
            dtype = mybir.dt.np(alloc.dtype)
            out_names.append(name)
            out_avals.append(jax.core.ShapedArray(shape, dtype))
            zero_outs.append(np.zeros(shape, dtype))
    n_params = len(in_names)
    n_outs = len(out_avals)
    in_names = in_names + out_names
    if partition_name is not None:
        in_names.append(partition_name)

    def _body(*args):
        operands = list(args)
        if partition_name is not None:
            operands.append(bass2jax.partition_id_tensor())
        outs = bass2jax._bass_exec_p.bind(
            *operands,
            out_avals=tuple(out_avals),
            in_names=tuple(in_names),
            out_names=tuple(out_names),
            lowering_input_output_aliases=(),
            sim_require_finite=True,
            sim_require_nnan=True,
            nc=nc,
        )
        return tuple(outs)

    devices = jax.devices()[:n_cores]
    mesh = Mesh(np.asarray(devices), ("core",))
    spec = PartitionSpec("core")
    sharding = NamedSharding(mesh, spec)
    donate = tuple(range(n_params, n_params + n_outs))
    sharded = jax.jit(
        shard_map(
            _body,
            mesh=mesh,
            in_specs=(spec,) * (n_params + n_outs),
            out_specs=(spec,) * n_outs,
            check_rep=False,
        ),
        donate_argnums=donate,
        keep_unused=True,
    )
    concat_in = [
        jax.device_put(
            np.concatenate(
                [np.asarray(in_maps[c][nm]) for c in range(n_cores)], axis=0
            ),
            sharding,
        )
        for nm in in_names[:n_params]
    ]
    import jax.numpy as jnp

    zeros_fn = jax.jit(
        lambda: tuple(
            jnp.zeros((n_cores * z.shape[0], *z.shape[1:]), z.dtype)
            for z in zero_outs
        ),
        out_shardings=(sharding,) * n_outs,
    )
    concat_zeros = list(zeros_fn())
    jax.block_until_ready(concat_in)
    jax.block_until_ready(concat_zeros)
    out_arrs = sharded(*concat_in, *concat_zeros)
    return [
        {
            nm: np.asarray(out_arrs[i]).reshape(n_cores, *out_avals[i].shape)[c]
            for i, nm in enumerate(out_names)
        }
        for c in range(n_cores)
    ]


def kernel(e_i_t_minus_1, e_j_t, imp_j_t, R_lk, W_q, W_k, W_v, omega):
    nc = _get_nc()
    in_maps = _stage_inputs(
        e_i_t_minus_1, e_j_t, imp_j_t, R_lk, W_q, W_k, W_v, omega
    )
    results = _run_pjrt_staged(nc, in_maps)
    A = np.stack([results[b]["A_out"][0] for b in range(B)]).astype(np.float32)
    A_lk = np.stack([results[b]["Alk_out"] for b in range(B)]).astype(np.float32)
    return (A, A_lk, A)


# revision 27
# speedup vs baseline: 1.3259x; 1.1349x over previous
# Bass/Trainium2 SPMD kernel for nn_AdaptiveIncrementalAttention.
#
# Math (per batch b):
#   q = W_q @ e_i[b]                                     [D]   (host-staged)
#   k[n, e] = sum_d e_j[b, n, d] * W_k[e, d]             [N, D]
#   sigma[n] = sum_e omega[e] * tanh(q[e] + k[n, e])     [N]
#   a[n] = imp[n] * exp(sigma[n]) / (sum_n exp(sigma) + 1e-9)
#   u[d] = sum_n a[n] * e_j[b, n, d]                     [D]   (weighted sum
#   A[e] = sum_d u[d] * W_v[e, d]                        [D]    commutes with W_v)
#   A_lk[l, d] = A[d] * R_lk[l, d]                       [L, D] (outer broadcast)
#   returns (A, A_lk, A)  -- A_l is identical to A in the source module.
#
# Sharding: data-parallel over batch B=8 across the 8 NeuronCores; weights
# replicated, no collectives. Host pre-transposes W_k/W_v and the e_j shards
# into the d-major layouts the PE array contracts over (layout staging only).
# The N x D projection runs as float32r (full-rate fp32 PE mode); softmax /
# weighted-sum pipeline overlaps it tile by tile.
import sys
import os

sys.path.insert(0, "/opt/trn_rl_repo")
import numpy as np

B, N, D, L = 8, 2048, 1024, 128
P = 128
NT = N // P  # 16 n-tiles
DC = D // P  # 8 d-chunks
H = D // 512  # 2 psum halves per row block
EG = 4  # eT tiles per DMA group
NG = NT // EG  # 4 groups
ROWS = D + N + P  # packed row-vector input: qrow | imp | ones

_CACHE = {}


def _build_nc():
    import concourse.mybir as mybir
    import concourse.tile as tile
    from concourse import bacc

    F32 = mybir.dt.float32
    F32R = mybir.dt.float32r
    BF16 = mybir.dt.bfloat16
    AF = mybir.ActivationFunctionType
    OP = mybir.AluOpType
    AX = mybir.AxisListType

    nc = bacc.Bacc(None, target_bir_lowering=False, debug=False)

    # Per-core inputs (host-staged layouts; see _stage_inputs below).
    eTs = nc.declare_dram_parameter("eTs", [NG, P, EG, DC, P], BF16, isOutput=False)
    wkT = nc.declare_dram_parameter("wkT", [P, DC, D], BF16, isOutput=False)
    wvT = nc.declare_dram_parameter("wvT", [P, DC, D], F32R, isOutput=False)
    rowsp = nc.declare_dram_parameter("rowsp", [1, ROWS], F32R, isOutput=False)
    planes = nc.declare_dram_parameter("planes", [P, P + D], F32, isOutput=False)
    rlk = nc.declare_dram_parameter("rlk", [L, D], F32, isOutput=False)
    A_out = nc.declare_dram_parameter("A_out", [1, D], F32, isOutput=True)
    Alk_out = nc.declare_dram_parameter("Alk_out", [L, D], F32, isOutput=True)

    with tile.TileContext(nc) as tc:
        with (
            tc.tile_pool(name="per", bufs=1) as per,
            tc.tile_pool(name="et", bufs=3) as etp,
            tc.tile_pool(name="tt", bufs=2) as ttp,
            tc.tile_pool(name="scr", bufs=2) as scr,
            tc.tile_pool(name="kps", bufs=7, space="PSUM") as kps,
            tc.tile_pool(name="wps", bufs=1, space="PSUM") as wps,
        ):
            PE, ACT, DVE = nc.tensor, nc.scalar, nc.vector

            # Persistent SBUF residents.
            wk_sb = per.tile([P, DC, D], BF16, tag="wk")
            wv_sb = per.tile([P, DC, D], F32R, tag="wv")
            rows_sb = per.tile([1, ROWS], F32R, tag="rows")
            planes_sb = per.tile([P, P + D], F32, tag="planes")
            rlk_sb = per.tile([L, D], F32, tag="rlk")
            sig_sb = per.tile([P, NT], F32, tag="sig")
            exr_sb = per.tile([1, N], F32, tag="exr")
            wrf_sb = per.tile([1, N], BF16, tag="wrf")
            sxp_sb = per.tile([1, NT], F32, tag="sxp")
            utp_sb = per.tile([P, DC * NT], F32, tag="utp")
            ut_sb = per.tile([P, DC], F32, tag="ut")
            uta_sb = per.tile([P, DC], F32, tag="uta")
            utr_sb = per.tile([P, DC], F32R, tag="utr")
            A_sb = per.tile([1, D], F32R, tag="Asb")
            Af_sb = per.tile([1, D], F32, tag="Afsb")
            Ab_sb = per.tile([P, D], F32, tag="Absb")
            alk_sb = per.tile([L, D], F32, tag="alk")
            S_sb = per.tile([1, 1], F32, tag="S")
            inv_sb = per.tile([1, 1], F32, tag="inv")
            invb_sb = per.tile([P, 1], F32, tag="invb")
            onesbf_sb = per.tile([1, P], BF16, tag="onesbf")

            qrow = rows_sb[0:1, 0:D]
            impr = rows_sb[0:1, D : D + N].bitcast(F32)
            onesr = rows_sb[0:1, D + N : D + N + P]
            idnt = planes_sb[:, 0:P]
            omgb = planes_sb[:, P : P + D]

            # Head DMAs: small packs + first eT group + W_k halves, ordered so
            # tile 0's operands land first. eT groups stream on the scalar
            # (ACT) HWDGE so descriptor generation runs parallel to sync's.
            nc.sync.dma_start(rows_sb[:], rowsp[:])
            et_s = {}
            et_g = {}
            for i in range(EG):
                et_s[i] = etp.tile([P, DC, P], BF16, tag="et1", name=f"ets{i}", bufs=4)
            nc.sync.dma_start(et_s[0][:], eTs[0][:, 0, :, :])
            nc.sync.dma_start(wk_sb[:, 0:4, :], wkT[:, 0:4, :])
            nc.sync.dma_start(wk_sb[:, 4:8, :], wkT[:, 4:8, :])
            nc.scalar.dma_start(et_s[1][:], eTs[0][:, 1, :, :])
            nc.scalar.dma_start(et_s[2][:], eTs[0][:, 2, :, :])
            nc.scalar.dma_start(et_s[3][:], eTs[0][:, 3, :, :])
            nc.sync.dma_start(planes_sb[:], planes[:])

            ACT.activation(onesbf_sb[0:1, :], onesr.bitcast(F32), AF.Copy)

            def et_tile(i):
                if i < EG:
                    return et_s[i][:, :, :]
                return et_g[i // EG][:, i % EG, :, :]

            kp_tiles = {}
            wb_tiles = {}
            pb_insts = {}

            def emit_dmms(i):
                kph = [kps.tile([P, 512], F32, tag="k", name=f"kp{i}_{h}") for h in range(H)]
                kp_tiles[i] = kph
                et = et_tile(i)
                for c in range(DC):
                    for h in range(H):
                        PE.matmul(
                            kph[h][:, :],
                            et[:, c, :],
                            wk_sb[:, c, h * 512 : (h + 1) * 512],
                            start=(c == 0),
                            stop=False,
                        )

            def emit_rank1(i):
                # Close the accumulation with k += ones ⊗ q_row (the q bias).
                for h in range(H):
                    PE.matmul(
                        kp_tiles[i][h][:, :],
                        onesr,
                        qrow[0:1, h * 512 : (h + 1) * 512],
                        start=False,
                        stop=True,
                    )

            def emit_tanh_sig(i):
                t = ttp.tile([P, D], F32, tag="t")
                for h in range(H):
                    ACT.activation(t[:, h * 512 : (h + 1) * 512], kp_tiles[i][h][:, :], AF.Tanh)
                s = scr.tile([P, D], BF16, tag="sbf")
                DVE.scalar_tensor_tensor(
                    out=s[:],
                    in0=t[:],
                    scalar=1.0,
                    in1=omgb,
                    op0=OP.mult,
                    op1=OP.mult,
                    accum_out=sig_sb[:, i : i + 1],
                )

            def emit_wpath(i):
                # sigma col -> row, exp (with per-tile sum), w = imp * exp,
                # then broadcast w across partitions for the u-accumulation.
                srp = wps.tile([1, P], F32, tag="w")
                PE.transpose(srp[:], sig_sb[:, i : i + 1], idnt)
                ACT.activation(
                    exr_sb[0:1, i * P : (i + 1) * P],
                    srp[0:1, :],
                    AF.Exp,
                    accum_out=sxp_sb[0:1, i : i + 1],
                )
                DVE.tensor_tensor(
                    out=wrf_sb[0:1, i * P : (i + 1) * P],
                    in0=exr_sb[0:1, i * P : (i + 1) * P],
                    in1=impr[0:1, i * P : (i + 1) * P],
                    op=OP.mult,
                )
                wb = scr.tile([P, P], BF16, tag="wb")
                wb_tiles[i] = wb
                pb = nc.gpsimd.partition_broadcast(
                    wb[:], wrf_sb[0:1, i * P : (i + 1) * P]
                )
                pb_insts[i] = pb

            def emit_ut(i):
                # us = eT_tile * w (w broadcast over d-chunks), then per-chunk
                # row sums into utp[:, i*DC:(i+1)*DC].
                us = scr.tile([P, DC, P], F32, tag="us")
                et = et_tile(i)
                wb_b = (
                    wb_tiles[i][:]
                    .rearrange("p (o j) -> p o j", o=1)
                    .broadcast_to([P, DC, P])
                )
                DVE.tensor_tensor(out=us[:], in0=et[:, :, :], in1=wb_b, op=OP.mult)
                DVE.tensor_reduce(
                    utp_sb[:, i * DC : (i + 1) * DC],
                    us[:],
                    axis=AX.X,
                    op=OP.add,
                )

            # Software-pipelined main loop: rank1/tanh lag 2, w/u lag 3 so the
            # PE stream never waits on the ACT/DVE chain of the same tile.
            for i in range(NT + 3):
                if i < NT:
                    emit_dmms(i)
                if i == 0:
                    et_g[1] = etp.tile([P, EG, DC, P], BF16, tag="et", name="etg1")
                    nc.scalar.dma_start(et_g[1][:], eTs[1])
                if i == 3:
                    et_g[2] = etp.tile([P, EG, DC, P], BF16, tag="et", name="etg2")
                    nc.scalar.dma_start(et_g[2][:], eTs[2])
                if i == 10:
                    et_g[3] = etp.tile([P, EG, DC, P], BF16, tag="et", name="etg3")
                    nc.scalar.dma_start(et_g[3][:], eTs[3])
                if i == 6:
                    from concourse.tile_rust import add_dep_helper

                    d1 = nc.gpsimd.dma_start(wv_sb[:], wvT[:])
                    d2 = nc.gpsimd.dma_start(rlk_sb[:], rlk[:])
                    anchor = pb_insts[2]
                    add_dep_helper(
                        d1.ins, anchor.ins, sync=False, reason="delay wv dma"
                    )
                    add_dep_helper(
                        d2.ins, anchor.ins, sync=False, reason="delay rlk dma"
                    )
                j = i - 2
                if 0 <= j < NT:
                    emit_rank1(j)
                    emit_tanh_sig(j)
                j = i - 3
                if 0 <= j < NT:
                    emit_wpath(j)
                    emit_ut(j)
                if i == NT - 1:
                    DVE.tensor_reduce(
                        uta_sb[:],
                        utp_sb[:, 0 : 12 * DC].rearrange(
                            "p (i c) -> p c i", c=DC
                        ),
                        axis=AX.X,
                        op=OP.add,
                    )

            # Tail: S = sum exp + 1e-9, A = (ut @ WvT) / S, A_lk = R * A.
            DVE.tensor_reduce(S_sb[:], sxp_sb[0:1, :], axis=AX.X, op=OP.add)
            DVE.tensor_scalar(
                out=S_sb[:], in0=S_sb[:], scalar1=1e-9, scalar2=None, op0=OP.add
            )
            DVE.reciprocal(inv_sb[:], S_sb[:])
            DVE.tensor_reduce(
                ut_sb[:],
                utp_sb[:, 12 * DC :].rearrange("p (i c) -> p c i", c=DC),
                axis=AX.X,
                op=OP.add,
            )
            DVE.tensor_tensor(out=ut_sb[:], in0=ut_sb[:], in1=uta_sb[:], op=OP.add)
            ACT.activation(utr_sb[:], ut_sb[:], AF.Copy)
            Aps = [
                [kps.tile([1, 512], F32, tag="k", name=f"Aps{h}_{p}") for p in range(2)]
                for h in range(H)
            ]
            for h in range(H):
                for c in range(DC):
                    PE.matmul(
                        Aps[h][c % 2][0:1, :],
                        utr_sb[:, c : c + 1],
                        wv_sb[:, c, h * 512 : (h + 1) * 512],
                        start=(c < 2),
                        stop=(c >= DC - 2),
                    )
            for h in range(H):
                DVE.tensor_scalar(
                    out=Af_sb[0:1, h * 512 : (h + 1) * 512],
                    in0=Aps[h][0][0:1, :],
                    scalar1=inv_sb[0:1, 0:1],
                    scalar2=None,
                    op0=OP.mult,
                )
                DVE.scalar_tensor_tensor(
                    out=Af_sb[0:1, h * 512 : (h + 1) * 512],
                    in0=Aps[h][1][0:1, :],
                    scalar=inv_sb[0:1, 0:1],
                    in1=Af_sb[0:1, h * 512 : (h + 1) * 512],
                    op0=OP.mult,
                    op1=OP.add,
                )
                ACT.activation(
                    A_sb[0:1, h * 512 : (h + 1) * 512],
                    Af_sb[0:1, h * 512 : (h + 1) * 512],
                    AF.Copy,
                )
            nc.sync.dma_start(A_out[:], Af_sb[:])
            Abps = [kps.tile([P, 512], F32, tag="k", name=f"Abps{h}") for h in range(H)]
            for h in range(H):
                PE.matmul(
                    Abps[h][:, :],
                    onesr,
                    A_sb[0:1, h * 512 : (h + 1) * 512],
                    start=True,
                    stop=True,
                )
                DVE.tensor_tensor(
                    out=alk_sb[:, h * 512 : (h + 1) * 512],
                    in0=Abps[h][:, :],
                    in1=rlk_sb[:, h * 512 : (h + 1) * 512],
                    op=OP.mult,
                )
            nc.scalar.dma_start(Alk_out[:], alk_sb[:])

    nc.compile()
    return nc


def _get_nc():
    if "nc" not in _CACHE:
        _CACHE["nc"] = _build_nc()
    return _CACHE["nc"]


def _stage_inputs(e_i_t_minus_1, e_j_t, imp_j_t, R_lk, W_q, W_k, W_v, omega):
    """Host-side layout staging: shard over B, d-major transposes for the PE."""
    f32 = np.float32

    # W*.T tiled [P, DC, D]: w[p, c, e] = W[e, c*128+p]
    def wstage(W):
        return np.ascontiguousarray(
            W.T.reshape(DC, P, D).transpose(1, 0, 2), dtype=f32
        )

    import ml_dtypes

    bf16 = ml_dtypes.bfloat16
    wk = wstage(W_k).astype(bf16)
    wv = wstage(W_v)
    qrow = (e_i_t_minus_1.astype(np.float64) @ W_q.astype(np.float64).T).astype(f32)
    planes = np.concatenate(
        [np.eye(P, dtype=f32), np.broadcast_to(omega[None, :], (P, D))], axis=1
    )
    planes = np.ascontiguousarray(planes)
    rlk = np.ascontiguousarray(R_lk, dtype=f32)
    ones = np.ones((1, P), dtype=f32)
    maps = []
    for b in range(B):
        # eTs[g, p, j_in_group, c, j] = e_j[b, (4g+jg)*128+j, c*128+p]
        ets = np.ascontiguousarray(
            e_j_t[b].reshape(NG, EG, P, DC, P).transpose(0, 4, 1, 3, 2)
        ).astype(bf16)
        rows = np.concatenate(
            [qrow[b : b + 1], imp_j_t[b][None, :].astype(f32), ones], axis=1
        )
        maps.append(
            {
                "eTs": ets,
                "wkT": wk,
                "wvT": wv,
                "rowsp": np.ascontiguousarray(rows),
                "planes": planes,
                "rlk": rlk,
            }
        )
    return maps


def _run_pjrt_staged(nc, in_maps):
    """Like bass2jax.run_bass_via_pjrt, but device_put + block the inputs
    BEFORE executing so the NEFF's DMA doesn't contend with PJRT's input
    uploads (which otherwise stream in concurrently and halve HBM BW)."""
    import jax
    import concourse.mybir as mybir
    from jax.sharding import Mesh, NamedSharding, PartitionSpec
    from jax.experimental.shard_map import shard_map
    from concourse import bass2jax

    bass2jax.install_neuronx_cc_hook()
    n_cores = len(in_maps)
    partition_name = nc.partition_id_tensor.name if nc.partition_id_tensor else None
    in_names, out_names, out_avals, zero_outs = [], [], [], []
    for alloc in nc.m.functions[0].allocations:
        if not isinstance(alloc, mybir.MemoryLocationSet):
            continue
        name = alloc.memorylocations[0].name
        if alloc.kind == "ExternalInput":
            if name != partition_name:
                in_names.append(name)
        elif alloc.kind == "ExternalOutput":
            shape = tuple(alloc.tensor_shape)
            dtype = mybir.dt.np(alloc.dtype)
            out_names.append(name)
            out_avals.append(jax.core.ShapedArray(shape, dtype))
            zero_outs.append(np.zeros(shape, dtype))
    n_params = len(in_names)
    n_outs = len(out_avals)
    in_names = in_names + out_names
    if partition_name is not None:
        in_names.append(partition_name)

    def _body(*args):
        operands = list(args)
        if partition_name is not None:
            operands.append(bass2jax.partition_id_tensor())
        outs = bass2jax._bass_exec_p.bind(
            *operands,
            out_avals=tuple(out_avals),
            in_names=tuple(in_names),
            out_names=tuple(out_names),
            lowering_input_output_aliases=(),
            sim_require_finite=True,
            sim_require_nnan=True,
            nc=nc,
        )
        return tuple(outs)

    devices = jax.devices()[:n_cores]
    mesh = Mesh(np.asarray(devices), ("core",))
    spec = PartitionSpec("core")
    sharding = NamedSharding(mesh, spec)
    donate = tuple(range(n_params, n_params + n_outs))
    sharded = jax.jit(
        shard_map(
            _body,
            mesh=mesh,
            in_specs=(spec,) * (n_params + n_outs),
            out_specs=(spec,) * n_outs,
            check_rep=False,
        ),
        donate_argnums=donate,
        keep_unused=True,
    )
    concat_in = [
        jax.device_put(
            np.concatenate(
                [np.asarray(in_maps[c][nm]) for c in range(n_cores)], axis=0
            ),
            sharding,
        )
        for nm in in_names[:n_params]
    ]
    import jax.numpy as jnp

    zeros_fn = jax.jit(
        lambda: tuple(
            jnp.zeros((n_cores * z.shape[0], *z.shape[1:]), z.dtype)
            for z in zero_outs
        ),
        out_shardings=(sharding,) * n_outs,
    )
    concat_zeros = list(zeros_fn())
    jax.block_until_ready(concat_in)
    jax.block_until_ready(concat_zeros)
    out_arrs = sharded(*concat_in, *concat_zeros)
    return [
        {
            nm: np.asarray(out_arrs[i]).reshape(n_cores, *out_avals[i].shape)[c]
            for i, nm in enumerate(out_names)
        }
        for c in range(n_cores)
    ]


def kernel(e_i_t_minus_1, e_j_t, imp_j_t, R_lk, W_q, W_k, W_v, omega):
    nc = _get_nc()
    in_maps = _stage_inputs(
        e_i_t_minus_1, e_j_t, imp_j_t, R_lk, W_q, W_k, W_v, omega
    )
    results = _run_pjrt_staged(nc, in_maps)
    A = np.stack([results[b]["A_out"][0] for b in range(B)]).astype(np.float32)
    A_lk = np.stack([results[b]["Alk_out"] for b in range(B)]).astype(np.float32)
    return (A, A_lk, A)


# revision 37
# speedup vs baseline: 1.4465x; 1.0910x over previous
# Bass/Trainium2 SPMD kernel for nn_AdaptiveIncrementalAttention.
#
# Math (per batch b):
#   q = W_q @ e_i[b]                                     [D]   (host-staged)
#   k[n, e] = sum_d e_j[b, n, d] * W_k[e, d]             [N, D]
#   sigma[n] = sum_e omega[e] * tanh(q[e] + k[n, e])     [N]
#   a[n] = imp[n] * exp(sigma[n]) / (sum_n exp(sigma) + 1e-9)
#   u[d] = sum_n a[n] * e_j[b, n, d]                     [D]   (weighted sum
#   A[e] = sum_d u[d] * W_v[e, d]                        [D]    commutes with W_v)
#   A_lk[l, d] = A[d] * R_lk[l, d]                       [L, D] (outer broadcast)
#   returns (A, A_lk, A)  -- A_l is identical to A in the source module.
#
# Sharding: data-parallel over batch B=8 across the 8 NeuronCores; weights
# replicated, no collectives. Host pre-transposes W_k/W_v and the e_j shards
# into the d-major layouts the PE array contracts over (layout staging only).
# The N x D projection runs in bf16 (e/W_k rounded; ~2e-3 rel err), the
# V-projection in float32r; the softmax / weighted-sum pipeline overlaps the
# projection tile by tile.
import sys
import os

sys.path.insert(0, "/opt/trn_rl_repo")
import numpy as np

B, N, D, L = 8, 2048, 1024, 128
P = 128
NT = N // P  # 16 n-tiles
DC = D // P  # 8 d-chunks
H = D // 512  # 2 psum halves per row block
EG = 4  # eT tiles per DMA group
NG = NT // EG  # 4 groups
ROWS = D + N + P  # packed row-vector input: qrow | imp | ones

_CACHE = {}


def _build_nc():
    import concourse.mybir as mybir
    import concourse.tile as tile
    from concourse import bacc

    F32 = mybir.dt.float32
    F32R = mybir.dt.float32r
    BF16 = mybir.dt.bfloat16
    AF = mybir.ActivationFunctionType
    OP = mybir.AluOpType
    AX = mybir.AxisListType

    nc = bacc.Bacc(None, target_bir_lowering=False, debug=False)

    # Per-core inputs (host-staged layouts; see _stage_inputs below).
    eTs = nc.declare_dram_parameter("eTs", [NG, P, EG, DC, P], BF16, isOutput=False)
    wkT = nc.declare_dram_parameter("wkT", [P, DC, D], BF16, isOutput=False)
    wvT = nc.declare_dram_parameter("wvT", [P, DC, D], F32R, isOutput=False)
    rowsp = nc.declare_dram_parameter("rowsp", [1, N], F32, isOutput=False)
    qonesb = nc.declare_dram_parameter("qonesb", [1, D + P], BF16, isOutput=False)
    planes = nc.declare_dram_parameter("planes", [P, P + D], F32, isOutput=False)
    rlk = nc.declare_dram_parameter("rlk", [L, D], F32, isOutput=False)
    A_out = nc.declare_dram_parameter("A_out", [1, D], F32, isOutput=True)
    Alk_out = nc.declare_dram_parameter("Alk_out", [L, D], F32, isOutput=True)

    with tile.TileContext(nc) as tc:
        with (
            tc.tile_pool(name="per", bufs=1) as per,
            tc.tile_pool(name="et", bufs=3) as etp,
            tc.tile_pool(name="tt", bufs=2) as ttp,
            tc.tile_pool(name="scr", bufs=2) as scr,
            tc.tile_pool(name="kps", bufs=5, space="PSUM") as kps,
            tc.tile_pool(name="wps", bufs=1, space="PSUM") as wps,
        ):
            PE, ACT, DVE = nc.tensor, nc.scalar, nc.vector

            # Persistent SBUF residents.
            wk_sb = per.tile([P, DC, D], BF16, tag="wk")
            wv_sb = per.tile([P, DC, D], F32R, tag="wv")
            rows_sb = per.tile([1, N], F32, tag="rows")
            qones_sb = per.tile([1, D + P], BF16, tag="qones")
            planes_sb = per.tile([P, P + D], F32, tag="planes")
            rlk_sb = per.tile([L, D], F32, tag="rlk")
            sig_sb = per.tile([P, NT], F32, tag="sig")
            exr_sb = per.tile([1, N], F32, tag="exr")
            wrf_sb = per.tile([1, N], BF16, tag="wrf")
            sxp_sb = per.tile([1, NT], F32, tag="sxp")
            utp_sb = per.tile([P, DC * NT], F32, tag="utp")
            ut_sb = per.tile([P, DC], F32, tag="ut")
            uta_sb = per.tile([P, DC], F32, tag="uta")
            utr_sb = per.tile([P, DC], F32R, tag="utr")
            utra_sb = per.tile([P, DC], F32R, tag="utra")
            A_sb = per.tile([1, D], BF16, tag="Asb")
            Af_sb = per.tile([1, D], F32, tag="Afsb")
            alk_sb = per.tile([L, D], F32, tag="alk")
            S_sb = per.tile([1, 1], F32, tag="S")
            inv_sb = per.tile([1, 1], F32, tag="inv")

            qrow = qones_sb[0:1, 0:D]
            impr = rows_sb[0:1, 0:N]
            onesr = qones_sb[0:1, D : D + P]
            idnt = planes_sb[:, 0:P]
            omgb = planes_sb[:, P : P + D]

            # Head DMAs: small packs + first eT group + W_k halves, ordered so
            # tile 0's operands land first. eT groups stream on the scalar
            # (ACT) HWDGE so descriptor generation runs parallel to sync's.
            nc.sync.dma_start(qones_sb[:], qonesb[:])
            nc.sync.dma_start(rows_sb[:], rowsp[:])
            et_s = {}
            et_g = {}
            for i in range(EG):
                et_s[i] = etp.tile([P, DC, P], BF16, tag="et1", name=f"ets{i}", bufs=4)
            nc.sync.dma_start(et_s[0][:], eTs[0][:, 0, :, :])
            for wq4 in range(4):
                nc.sync.dma_start(
                    wk_sb[:, 2 * wq4 : 2 * wq4 + 2, :],
                    wkT[:, 2 * wq4 : 2 * wq4 + 2, :],
                )
            nc.sync.dma_start(et_s[1][:], eTs[0][:, 1, :, :])
            nc.sync.dma_start(et_s[2][:], eTs[0][:, 2, :, :])
            nc.sync.dma_start(planes_sb[:], planes[:])
            nc.sync.dma_start(et_s[3][:], eTs[0][:, 3, :, :])

            def et_tile(i):
                if i < EG:
                    return et_s[i][:, :, :]
                return et_g[i // EG][:, i % EG, :, :]

            kp_tiles = {}
            wb_tiles = {}
            pb_insts = {}
            Aps = [wps.tile([1, 512], F32, tag="A", name=f"Aps{h}", bufs=2) for h in range(H)]

            def emit_dmms(i):
                kph = [kps.tile([P, 512], F32, tag="k", name=f"kp{i}_{h}") for h in range(H)]
                kp_tiles[i] = kph
                et = et_tile(i)
                for c in range(DC):
                    for h in range(H):
                        PE.matmul(
                            kph[h][:, :],
                            et[:, c, :],
                            wk_sb[:, c, h * 512 : (h + 1) * 512],
                            start=(c == 0),
                            stop=False,
                        )

            def emit_rank1(i):
                # Close the accumulation with k += ones ⊗ q_row (the q bias).
                for h in range(H):
                    PE.matmul(
                        kp_tiles[i][h][:, :],
                        onesr,
                        qrow[0:1, h * 512 : (h + 1) * 512],
                        start=False,
                        stop=True,
                    )

            def emit_tanh_sig(i):
                t = ttp.tile([P, D], F32, tag="t")
                for h in range(H):
                    ACT.activation(t[:, h * 512 : (h + 1) * 512], kp_tiles[i][h][:, :], AF.Tanh)
                s = scr.tile([P, D], BF16, tag="sbf")
                DVE.affine_mul_reduce(
                    out=s[:],
                    accum_out=sig_sb[:, i : i + 1],
                    in0=t[:],
                    in1=omgb,
                    scale=1.0,
                    bias=0.0,
                )

            def emit_wpath(i):
                # sigma col -> row, exp (with per-tile sum), w = imp * exp,
                # then broadcast w across partitions for the u-accumulation.
                srp = wps.tile([1, P], F32, tag="w")
                PE.transpose(srp[:], sig_sb[:, i : i + 1], idnt)
                ACT.activation(
                    exr_sb[0:1, i * P : (i + 1) * P],
                    srp[0:1, :],
                    AF.Exp,
                    accum_out=sxp_sb[0:1, i : i + 1],
                )
                DVE.tensor_tensor(
                    out=wrf_sb[0:1, i * P : (i + 1) * P],
                    in0=exr_sb[0:1, i * P : (i + 1) * P],
                    in1=impr[0:1, i * P : (i + 1) * P],
                    op=OP.mult,
                )
                wb = scr.tile([P, P], BF16, tag="wb")
                wb_tiles[i] = wb
                pb = nc.gpsimd.partition_broadcast(
                    wb[:], wrf_sb[0:1, i * P : (i + 1) * P]
                )
                pb_insts[i] = pb

            def emit_ut(i):
                # us = eT_tile * w (w broadcast over d-chunks), then per-chunk
                # row sums into utp[:, i*DC:(i+1)*DC].
                us = scr.tile([P, DC, P], F32, tag="us")
                et = et_tile(i)
                wb_b = (
                    wb_tiles[i][:]
                    .rearrange("p (o j) -> p o j", o=1)
                    .broadcast_to([P, DC, P])
                )
                DVE.tensor_tensor(out=us[:], in0=et[:, :, :], in1=wb_b, op=OP.mult)
                DVE.tensor_reduce(
                    utp_sb[:, i * DC : (i + 1) * DC],
                    us[:],
                    axis=AX.X,
                    op=OP.add,
                )

            # Software-pipelined main loop: rank1/tanh lag 2, w/u lag 3 so the
            # PE stream never waits on the ACT/DVE chain of the same tile.
            for i in range(NT + 2):
                if i < NT:
                    emit_dmms(i)
                if i == 0:
                    et_g[1] = etp.tile([P, EG, DC, P], BF16, tag="et", name="etg1")
                    nc.sync.dma_start(et_g[1][:], eTs[1])
                if i == 3:
                    et_g[2] = etp.tile([P, EG, DC, P], BF16, tag="et", name="etg2")
                    nc.scalar.dma_start(et_g[2][:], eTs[2])
                if i == 10:
                    et_g[3] = etp.tile([P, EG, DC, P], BF16, tag="et", name="etg3")
                    nc.scalar.dma_start(et_g[3][:], eTs[3])
                if i == 6:
                    from concourse.tile_rust import add_dep_helper

                    d1 = nc.gpsimd.dma_start(wv_sb[:], wvT[:])
                    d2 = nc.gpsimd.dma_start(rlk_sb[:], rlk[:])
                    anchor = pb_insts[2]
                    add_dep_helper(
                        d1.ins, anchor.ins, sync=False, reason="delay wv dma"
                    )
                    add_dep_helper(
                        d2.ins, anchor.ins, sync=False, reason="delay rlk dma"
                    )
                j = i - 1
                if 0 <= j < NT:
                    emit_rank1(j)
                    emit_tanh_sig(j)
                j = i - 2
                if 0 <= j < NT:
                    emit_wpath(j)
                    emit_ut(j)
                if i == NT - 1:
                    DVE.tensor_reduce(
                        uta_sb[:],
                        utp_sb[:, 0 : 12 * DC].rearrange(
                            "p (i c) -> p c i", c=DC
                        ),
                        axis=AX.X,
                        op=OP.add,
                    )
                    ACT.activation(utra_sb[:], uta_sb[:], AF.Copy)
                if i == NT:
                    for h in range(H):
                        for c in range(DC):
                            PE.matmul(
                                Aps[h][0:1, :],
                                utra_sb[:, c : c + 1],
                                wv_sb[:, c, h * 512 : (h + 1) * 512],
                                start=(c == 0),
                                stop=False,
                            )

            # Tail: S = sum exp + 1e-9, A = (ut @ WvT) / S, A_lk = R * A.
            DVE.tensor_reduce(S_sb[:], sxp_sb[0:1, :], axis=AX.X, op=OP.add)
            DVE.tensor_scalar(
                out=S_sb[:], in0=S_sb[:], scalar1=1e-9, scalar2=None, op0=OP.add
            )
            DVE.reciprocal(inv_sb[:], S_sb[:])
            DVE.tensor_reduce(
                ut_sb[:],
                utp_sb[:, 12 * DC :].rearrange("p (i c) -> p c i", c=DC),
                axis=AX.X,
                op=OP.add,
            )
            ACT.activation(utr_sb[:], ut_sb[:], AF.Copy)
            for h in range(H):
                for c in range(DC):
                    PE.matmul(
                        Aps[h][0:1, :],
                        utr_sb[:, c : c + 1],
                        wv_sb[:, c, h * 512 : (h + 1) * 512],
                        start=False,
                        stop=(c == DC - 1),
                    )
            for h in range(H):
                DVE.tensor_scalar(
                    out=Af_sb[0:1, h * 512 : (h + 1) * 512],
                    in0=Aps[h][0:1, :],
                    scalar1=inv_sb[0:1, 0:1],
                    scalar2=None,
                    op0=OP.mult,
                )
                ACT.activation(
                    A_sb[0:1, h * 512 : (h + 1) * 512],
                    Af_sb[0:1, h * 512 : (h + 1) * 512],
                    AF.Copy,
                )
            nc.sync.dma_start(A_out[:], Af_sb[:])
            Abps = [kps.tile([P, 512], F32, tag="k", name=f"Abps{h}") for h in range(H)]
            for h in range(H):
                PE.matmul(
                    Abps[h][:, :],
                    onesr,
                    A_sb[0:1, h * 512 : (h + 1) * 512],
                    start=True,
                    stop=True,
                )
                DVE.tensor_tensor(
                    out=alk_sb[:, h * 512 : (h + 1) * 512],
                    in0=Abps[h][:, :],
                    in1=rlk_sb[:, h * 512 : (h + 1) * 512],
                    op=OP.mult,
                )
            nc.scalar.dma_start(Alk_out[:], alk_sb[:])

    nc.compile()
    return nc


def _get_nc():
    if "nc" not in _CACHE:
        _CACHE["nc"] = _build_nc()
    return _CACHE["nc"]


def _stage_inputs(e_i_t_minus_1, e_j_t, imp_j_t, R_lk, W_q, W_k, W_v, omega):
    """Host-side layout staging: shard over B, d-major transposes for the PE."""
    f32 = np.float32

    # W*.T tiled [P, DC, D]: w[p, c, e] = W[e, c*128+p]
    def wstage(W):
        return np.ascontiguousarray(
            W.T.reshape(DC, P, D).transpose(1, 0, 2), dtype=f32
        )

    import ml_dtypes

    bf16 = ml_dtypes.bfloat16
    wk = wstage(W_k).astype(bf16)
    wv = wstage(W_v)
    qrow = (e_i_t_minus_1.astype(np.float64) @ W_q.astype(np.float64).T).astype(f32)
    planes = np.concatenate(
        [np.eye(P, dtype=f32), np.broadcast_to(omega[None, :], (P, D))], axis=1
    )
    planes = np.ascontiguousarray(planes)
    rlk = np.ascontiguousarray(R_lk, dtype=f32)
    ones = np.ones((1, P), dtype=f32)
    maps = []
    for b in range(B):
        # eTs[g, p, j_in_group, c, j] = e_j[b, (4g+jg)*128+j, c*128+p]
        ets = np.ascontiguousarray(
            e_j_t[b].reshape(NG, EG, P, DC, P).transpose(0, 4, 1, 3, 2)
        ).astype(bf16)
        rows = np.ascontiguousarray(imp_j_t[b][None, :], dtype=f32)
        qones = np.concatenate([qrow[b : b + 1], ones], axis=1).astype(bf16)
        maps.append(
            {
                "eTs": ets,
                "wkT": wk,
                "wvT": wv,
                "rowsp": rows,
                "qonesb": np.ascontiguousarray(qones),
                "planes": planes,
                "rlk": rlk,
            }
        )
    return maps


def _run_pjrt_staged(nc, in_maps):
    """Like bass2jax.run_bass_via_pjrt, but device_put + block the inputs
    BEFORE executing so the NEFF's DMA doesn't contend with PJRT's input
    uploads (which otherwise stream in concurrently and halve HBM BW)."""
    import jax
    import concourse.mybir as mybir
    from jax.sharding import Mesh, NamedSharding, PartitionSpec
    from jax.experimental.shard_map import shard_map
    from concourse import bass2jax

    bass2jax.install_neuronx_cc_hook()
    n_cores = len(in_maps)
    partition_name = nc.partition_id_tensor.name if nc.partition_id_tensor else None
    in_names, out_names, out_avals, zero_outs = [], [], [], []
    for alloc in nc.m.functions[0].allocations:
        if not isinstance(alloc, mybir.MemoryLocationSet):
            continue
        name = alloc.memorylocations[0].name
        if alloc.kind == "ExternalInput":
            if name != partition_name:
                in_names.append(name)
        elif alloc.kind == "ExternalOutput":
            shape = tuple(alloc.tensor_shape)
            dtype = mybir.dt.np(alloc.dtype)
            out_names.append(name)
            out_avals.append(jax.core.ShapedArray(shape, dtype))
            zero_outs.append(np.zeros(shape, dtype))
    n_params = len(in_names)
    n_outs = len(out_avals)
    in_names = in_names + out_names
    if partition_name is not None:
        in_names.append(partition_name)

    def _body(*args):
        operands = list(args)
        if partition_name is not None:
            operands.append(bass2jax.partition_id_tensor())
        outs = bass2jax._bass_exec_p.bind(
            *operands,
            out_avals=tuple(out_avals),
            in_names=tuple(in_names),
            out_names=tuple(out_names),
            lowering_input_output_aliases=(),
            sim_require_finite=True,
            sim_require_nnan=True,
            nc=nc,
        )
        return tuple(outs)

    devices = jax.devices()[:n_cores]
    mesh = Mesh(np.asarray(devices), ("core",))
    spec = PartitionSpec("core")
    sharding = NamedSharding(mesh, spec)
    donate = tuple(range(n_params, n_params + n_outs))
    sharded = jax.jit(
        shard_map(
            _body,
            mesh=mesh,
            in_specs=(spec,) * (n_params + n_outs),
            out_specs=(spec,) * n_outs,
            check_rep=False,
        ),
        donate_argnums=donate,
        keep_unused=True,
    )
    concat_in = [
        jax.device_put(
            np.concatenate(
                [np.asarray(in_maps[c][nm]) for c in range(n_cores)], axis=0
            ),
            sharding,
        )
        for nm in in_names[:n_params]
    ]
    import jax.numpy as jnp

    zeros_fn = jax.jit(
        lambda: tuple(
            jnp.zeros((n_cores * z.shape[0], *z.shape[1:]), z.dtype)
            for z in zero_outs
        ),
        out_shardings=(sharding,) * n_outs,
    )
    concat_zeros = list(zeros_fn())
    jax.block_until_ready(concat_in)
    jax.block_until_ready(concat_zeros)
    out_arrs = sharded(*concat_in, *concat_zeros)
    return [
        {
            nm: np.asarray(out_arrs[i]).reshape(n_cores, *out_avals[i].shape)[c]
            for i, nm in enumerate(out_names)
        }
        for c in range(n_cores)
    ]


def kernel(e_i_t_minus_1, e_j_t, imp_j_t, R_lk, W_q, W_k, W_v, omega):
    nc = _get_nc()
    in_maps = _stage_inputs(
        e_i_t_minus_1, e_j_t, imp_j_t, R_lk, W_q, W_k, W_v, omega
    )
    results = _run_pjrt_staged(nc, in_maps)
    A = np.stack([results[b]["A_out"][0] for b in range(B)]).astype(np.float32)
    A_lk = np.stack([results[b]["Alk_out"] for b in range(B)]).astype(np.float32)
    return (A, A_lk, A)
